# revision 33
# baseline (speedup 1.0000x reference)
"""Distributed sparse-attention kernel for 8 TRN2 NeuronCores.

Sharding: Megatron-style head parallelism. Core c owns heads [4c, 4c+4):
Wq/Wk/Wv column-parallel, Wo row-parallel. Each core computes a partial
output out_c = Wo_c @ ctx_c over its heads; the host sums the 8 partials.

Software-pipelined schedule (the Tile scheduler is priority+dependency
driven, so emission order ~ schedule):
  S0: b0 projections (K, V, Q).  DMA queue order tracks consumption
      order: wk+hkv interleaved (first chunks split finer so the first
      matmul's deps land early), then wv, hq, wq, wo.
  S1: b0 attention (qb-major segments) || b1 projections.  The proj
      filler is BACKLOADED (drain 1/iter for j<16, 2/iter after): late
      S1 has no other PE slack and the score->add->exp->ctx chain
      stalls would surface on the PE.
  S2: b1 attention || b0 out-projection (1 unit/iter, DVE evac only)
      || b1 qb0 out-projection (from j=19 == norm+2, evacs
      alternating DVE/ACT).
  S3: final norm + b1 qb1 out-projection in psc-paired tail units,
      interleaved with the 4 held-back o1q0 units as norm-latency
      filler.

Attention per (hp, qb, kc) iteration: scores for the head pair land in
ONE [128,1024] PSUM pair; zero-padded per-head K/Q operands keep every
matmul full C=128 geometry (uniform LDWEIGHTS stays pipelined).  pbm
(position_bias+mask) added on DVE for b0 / identity-matmul on PE for
b1.  One [128,1024] exp on ACT per iteration.  ctx matmuls trail by
CTX_LAG=2 ACROSS segment boundaries (head-of-line blocking in the
in-order PE queue otherwise).  hvT ones-column LAST per head: ctx psum
row 64 is the softmax denominator; normalize = ACT Ln + ACT Exp(-x) +
gpsimd partition_broadcast + 2 DVE muls.

HARD-WON CONSTRAINTS (all verified on HW this/prev session; violating
any costs 3-11us):
 - The combined ln+exp ACT table (natural_log_exp_and_others) is
   preloaded via an explicit InstLoadActFuncSet BEFORE any activation;
   the rust table pass respects pre-placed loads -> exactly 1
   ACT_TABLE_LOAD in the whole kernel (was 17, -16us).  Do NOT replace
   the Ln/Exp reciprocal with nc.vector.reciprocal: single-partition
   [1,N] DVE ops are serial (~1.4us each) and sit on the critical DVE
   queue (-37us regression).
 - GPSIMD cannot access PSUM (BIR verifier rejects) -> only DVE ("v")
   and ACT ("s") can evacuate psum.
 - ACT has ZERO slack while an attention phase runs: putting b0
   out-proj evacs on ACT in S2 = +6us.  The cxr evacs (segment-end
   cadence) and o1q0's alternating halves (land near S2-end) are the
   proven exceptions.
 - Consecutive accumulating matmuls into the SAME psum bank region
   stall ~250ns each (379->630ns): emit c-outer / bank-alternating
   (see outproj_tail).
 - o1q0 must start at exactly 2*NKC+CTX_LAG+2 (j=22 was +11us); the
   last-segment ctx-lag collapse was -10us (scheduler butterfly);
   PBM_LOOKAHEAD=7 was +4us.  The schedule is a sharp local optimum:
   test EVERY change, one at a time.

Precision: Q/K path fp16, V/ctx/out path bf16, tmq f32. fp8 anywhere
fails the 2e-2 rel-err budget (analysis: scores need <0.01 abs err;
V/out path fp8 adds ~2.5% rel).  Known-broken on HW:
reciprocal_approx_fast, partition_broadcast from non-zero base
partition.

Measurement notes: the DEVICE randomly sits in a ~2.0GHz power state
for whole processes -- check NTFF matmul median: 379ns good / 454ns
bad, scale by 1.2 before comparing.  Good-state noise +-1.5us.
Best this session: 215145ns (baseline was 237866).  Remaining idle at
best config: startup ~8us (NOP barrier + cold DMA), S2/S3 stutter
~8us (ppj 2-buf rotation + final-norm chain ~5us), post-PE out-DMA
drain ~6us (2MB of qb1 output is norm-gated to the last ~12us).
Do NOT schedule the hvT (V) evacuations close to their ctx-matmul
consumers: first-run read-before-write race (silently corrupts only
the first execution).
"""

import sys

for _p in ("/opt/trn_rl_repo",):
    if _p not in sys.path:
        sys.path.insert(0, _p)

from contextlib import ExitStack, nullcontext

import numpy as np
import ml_dtypes

import concourse.bass as bass
import concourse.mybir as mybir
import concourse.tile as tile
from concourse import bacc
from concourse.bass_utils import run_bass_kernel_spmd

B, D, H, DH, LQ, LK = 2, 2048, 32, 64, 1024, 1024
NCORES = 8
HC = H // NCORES          # heads per core = 4
HP = HC // 2              # head pairs = 2
MR = HC * DH              # per-core model rows = 256
NEG = -1e30

DC = D // 128             # 16 d-chunks
NKC = LK // 128           # 8 k-chunks
NQB = LQ // 512           # 2 q blocks
NMC = MR // 128           # 2 dh-chunks (== HP)
NOC = D // 128            # 16 output-row chunks
DHP = DH + 1              # 65: ones-column + 64 v-rows

F32 = mybir.dt.float32
F16 = mybir.dt.float16
BF16 = mybir.dt.bfloat16

_NP = {F32: np.float32, F16: np.float16, BF16: ml_dtypes.bfloat16}

CROSSBANK_ADD = True      # single [128,1024] DVE add across 2 psum banks
PBM_LOOKAHEAD = 6
PROJ_PER_ITER = 1         # b1 proj units emitted per b0 attention iter
CTX_LAG = 2               # iterations the ctx matmuls trail the exp stream


def build_nc():
    nc = bacc.Bacc("TRN2", target_bir_lowering=False, debug=False,
                   num_devices=NCORES)
    hq_e = nc.declare_dram_parameter("hq", [B, DC // 2, 128, 2 * LQ], F16,
                                     False)
    hkv_e = nc.declare_dram_parameter("hkv", [B, DC // 2, 128, 2 * LK], F16,
                                      False)
    pbm_e = nc.declare_dram_parameter("pbm", [B, HP, NKC, NQB, 128, 1024],
                                      BF16, False)
    wqt_e = nc.declare_dram_parameter("wqt", [DC // 4, 128, 4 * MR], F16,
                                      False)
    wkt_e = nc.declare_dram_parameter("wkt", [DC // 4, 128, 4 * MR], F16,
                                      False)
    wvt_e = nc.declare_dram_parameter("wvt", [DC // 4, 128, 4 * MR], F16,
                                      False)
    wot_e = nc.declare_dram_parameter("wot", [MR, D], BF16, False)
    id_e = nc.declare_dram_parameter("ident", [128, 128], BF16, False)
    out_e = nc.declare_dram_parameter("out", [B, D, LQ], BF16, True)
    hq_a, hkv_a, pbm_a, out_a = hq_e.ap(), hkv_e.ap(), pbm_e.ap(), out_e.ap()

    mm = nc.tensor.matmul
    Exp = mybir.ActivationFunctionType.Exp
    Ln = mybir.ActivationFunctionType.Ln

    # preload the ACT table that holds BOTH exp and ln so the normalize
    # clusters (Ln + Exp) never swap tables mid-kernel
    from concourse.hw_specs import get_activation_tables
    _tabs = list(get_activation_tables(nc.m.arch).items())
    _set_id = next(i for i, (_n, _fs) in enumerate(_tabs)
                   if Exp in _fs and Ln in _fs)

    with tile.TileContext(nc) as tc, ExitStack() as ctx:
        _ld = mybir.InstLoadActFuncSet(
            name=f"I-{nc.next_id()}", ins=[], outs=[])
        _ld.act_func_set_id = _set_id
        nc.scalar.add_instruction(_ld)
        wp = ctx.enter_context(tc.tile_pool(name="w", bufs=1))
        hidp = ctx.enter_context(tc.tile_pool(name="hid", bufs=1))
        sbp = ctx.enter_context(tc.tile_pool(name="sb", bufs=2))
        pbmp = ctx.enter_context(tc.tile_pool(name="pbm", bufs=7))
        tmqp = ctx.enter_context(tc.tile_pool(name="tmq", bufs=2))
        expp = ctx.enter_context(tc.tile_pool(name="ex", bufs=4))
        cxp = ctx.enter_context(tc.tile_pool(name="cx", bufs=4))
        nrmp = ctx.enter_context(tc.tile_pool(name="nrm", bufs=2))
        osbp = ctx.enter_context(tc.tile_pool(name="osb", bufs=3))
        # PSUM: scores 2x[128,1024] (4 banks) + ctx 2x[65,512] (2 banks)
        # + proj/outproj 2x[128,512] (2 banks) = 8 banks exactly
        psc = ctx.enter_context(tc.tile_pool(name="psc", bufs=2,
                                             space="PSUM"))
        pcx = ctx.enter_context(tc.tile_pool(name="pcx", bufs=2,
                                             space="PSUM"))
        ppj = ctx.enter_context(tc.tile_pool(name="ppj", bufs=2,
                                             space="PSUM"))

        # --- persistent weights ------------------------------------------
        def load_w4(nm, e):
            views = []
            for g in range(DC // 4):
                t = wp.tile([128, 4 * MR], F16, tag=f"{nm}{g}",
                            name=f"{nm}{g}")
                nc.sync.dma_start(t[:, :], e.ap()[g])
                for i in range(4):
                    views.append(t[:, i * MR:(i + 1) * MR])
            return views

        def load_w1(nm, e, g):
            t = wp.tile([128, 4 * MR], F16, tag=f"{nm}{g}", name=f"{nm}{g}")
            nc.sync.dma_start(t[:, :], e.ap()[g])
            return [t[:, i * MR:(i + 1) * MR] for i in range(4)]

        # hidden tiles: all 8 d-chunk-pairs of a batch stay resident while
        # that batch's proj groups run (each group loops over all dc)
        def fetch_hid(kind, a, b):
            ts = []
            for dc2 in range(DC // 2):
                t = hidp.tile([128, 2 * LK], F16, tag=f"{kind}{dc2}",
                              name=f"{kind}{b}_{dc2}")
                nc.sync.dma_start(t[:, :], a[b, dc2])
                ts.append(t)
            return ts

        # startup DMA order interleaves weight chunks with the hidden
        # stream each projection consumes, in consumption order; the very
        # first chunks are split finer so the first K-proj matmul's
        # dependencies (wk quarter 0 + hkv dc 0) land ~2us earlier
        wk0 = wp.tile([128, 4 * MR], F16, tag="wk0", name="wk0")
        nc.sync.dma_start(wk0[:, 0:MR], wkt_e.ap()[0][:, 0:MR])
        hkv_t0 = []
        t0h = hidp.tile([128, 2 * LK], F16, tag="hkvh0", name="hkvh0_0")
        nc.sync.dma_start(t0h[:, 0:LK], hkv_a[0, 0][:, 0:LK])
        nc.sync.dma_start(wk0[:, MR:], wkt_e.ap()[0][:, MR:])
        nc.sync.dma_start(t0h[:, LK:], hkv_a[0, 0][:, LK:])
        hkv_t0.append(t0h)
        wk_v = [wk0[:, i * MR:(i + 1) * MR] for i in range(4)]
        for dc2 in range(1, DC // 2):
            t = hidp.tile([128, 2 * LK], F16, tag=f"hkvh{dc2}",
                          name=f"hkvh0_{dc2}")
            nc.sync.dma_start(t[:, :], hkv_a[0, dc2])
            hkv_t0.append(t)
            if dc2 in (1, 3, 5):
                wk_v += load_w1("wk", wkt_e, (dc2 + 1) // 2)
        ident = wp.tile([128, 128], BF16, tag="ident", name="ident")
        nc.sync.dma_start(ident[:, :], id_e.ap()[:, :])
        # DMA queue order tracks consumption order: V runs BEFORE Q in
        # S0, so wv must precede wq (a 2.8us V stall otherwise); hq
        # streams during V so it precedes wq too
        wv_v = load_w4("wv", wvt_e)
        hq_t0 = fetch_hid("hqh", hq_a, 0)
        wq_v = load_w4("wq", wqt_e)

        # persistent hvT tiles, ones-column FIRST per head
        hvT = {}
        for b in range(B):
            for kc in range(NKC):
                t = wp.tile([128, HC * DHP], BF16, tag=f"hv{kc}_{b}",
                            name=f"hv{kc}_{b}")
                hvT[(b, kc)] = t
        for b in range(B):
            for kc in range(NKC):
                for h in range(HC):
                    nc.gpsimd.memset(
                        hvT[(b, kc)][:, h * DHP + DH:(h + 1) * DHP], 1.0)

        hk_sb, hq_sb, ctxn = {}, {}, {}
        wo_v = []

        def load_wo():
            for c in range(NMC):
                t = wp.tile([128, D], BF16, tag=f"wo{c}", name=f"wo{c}")
                nc.sync.dma_start(t[:, :],
                                  wot_e.ap()[c * 128:(c + 1) * 128, :])
                wo_v.append(t)

        def hid_view(ts, dc):
            return ts[dc // 2][:, (dc % 2) * LK:(dc % 2) * LK + LK]

        def alloc_hkhq(b):
            # per-head zero-padded score operands: head h keeps its data in
            # the same partition rows as the packed projection output
            # (even h: rows 0-63, odd h: rows 64-127), other rows zero, so
            # score matmuls are full C=128 geometry (uniform with all other
            # matmuls -> LDWEIGHTS stays pipelined) with no partition moves
            for h in range(HC):
                hk_sb[(b, h)] = sbp.tile([128, LK], F16, tag=f"hkp{h}",
                                         name=f"hkp{b}_{h}")
                hq_sb[(b, h)] = sbp.tile([128, LQ], F16, tag=f"hqp{h}",
                                         name=f"hqp{b}_{h}")
                z = slice(64, 128) if h % 2 == 0 else slice(0, 64)
                nc.gpsimd.memset(hk_sb[(b, h)][z, :], 0.0)
                nc.gpsimd.memset(hq_sb[(b, h)][z, :], 0.0)

        def v_units(b, hkv_ts):
            # V (transposed): bank holds [k=128, MR..] for a kc pair
            for g in range(NKC // 2):
                pj = ppj.tile([128, 512], F32, tag="pj", name=f"pv{b}_{g}")
                for dc in range(DC):
                    for i in range(2):
                        kc = 2 * g + i
                        mm(pj[:, i * MR:(i + 1) * MR],
                           hid_view(hkv_ts, dc)[:, kc * 128:(kc + 1) * 128],
                           wv_v[dc][:, :],
                           start=(dc == 0 and i == 0),
                           stop=(dc == DC - 1 and i == 1))
                    if dc % 4 == 3:
                        yield
                for i in range(2):
                    kc = 2 * g + i
                    for h in range(HC):
                        nc.vector.tensor_copy(
                            hvT[(b, kc)][:, h * DHP:h * DHP + DH],
                            pj[:, i * MR + h * DH:i * MR + (h + 1) * DH])
                    yield

        def proj_b0_dc_outer(nm, wv_, ts, dst):
            """b0's K/Q: dc-outer accumulation in the (idle at S0) score
            psum pairs so hidden-DMA and PE rate-match from the start."""
            ps = [psc.tile([128, 1024], F32, tag="sc", name=f"p{nm}0_{mc}")
                  for mc in range(NMC)]
            for dc in range(DC):
                for mc in range(NMC):
                    for kb in range(2):
                        mm(ps[mc][:, kb * 512:(kb + 1) * 512],
                           wv_[dc][:, mc * 128:(mc + 1) * 128],
                           hid_view(ts, dc)[:, kb * 512:(kb + 1) * 512],
                           start=dc == 0, stop=dc == DC - 1)
            for mc in range(NMC):
                nc.vector.tensor_copy(dst[(0, 2 * mc)][0:64, :],
                                      ps[mc][0:64, :])
                nc.vector.tensor_copy(dst[(0, 2 * mc + 1)][64:128, :],
                                      ps[mc][64:128, :])

        # --- b1 projection emission units (group style, 2 pj banks) ------
        def kq_units(b, nm, wv_, ts, dst):
            for mc in range(NMC):
                for kb in range(2):
                    pj = ppj.tile([128, 512], F32, tag="pj",
                                  name=f"p{nm}{b}_{mc}_{kb}")
                    for dc in range(DC):
                        mm(pj[:, :], wv_[dc][:, mc * 128:(mc + 1) * 128],
                           hid_view(ts, dc)[:, kb * 512:(kb + 1) * 512],
                           start=dc == 0, stop=dc == DC - 1)
                        if dc % 8 == 7:
                            yield
                    s = slice(kb * 512, (kb + 1) * 512)
                    nc.vector.tensor_copy(dst[(b, 2 * mc)][0:64, s],
                                          pj[0:64, :])
                    nc.vector.tensor_copy(dst[(b, 2 * mc + 1)][64:128, s],
                                          pj[64:128, :])
                    yield

        def proj_units(b, hkv_ts):
            # V no later than between K and Q: the hvT writes must land
            # well before the ctx matmuls that read them (keeps the
            # scheduler far from the observed first-run hvT read race)
            alloc_hkhq(b)
            yield from kq_units(b, "k", wk_v, hkv_ts, hk_sb)
            yield from v_units(b, hkv_ts)
            hq_ts = fetch_hid("hqh", hq_a, b)
            yield from kq_units(b, "q", wq_v, hq_ts, hq_sb)

        def drain(it, n=None):
            k = 0
            for _ in it:
                k += 1
                if n is not None and k >= n:
                    return False
            return True

        # --- attention ----------------------------------------------------
        cxr = {}

        def attn_iters(b):
            """Yield once per (hp, qb, kc) iteration."""
            pre = {}

            def fetch_pbm(hp, qb, kc):
                t = pbmp.tile([128, 1024], BF16, tag="pbm",
                              name=f"pbm{b}_{hp}_{qb}_{kc}")
                nc.sync.dma_start(t[:, :], pbm_a[b, hp, kc, qb])
                pre[(hp, qb, kc)] = t

            # qb-major: both head-pairs' qb0 cxr are ready by iteration
            # 2*NKC+CTX_LAG-1, so qb0 normalize + qb0 out-projection can
            # overlap the qb1 attention half
            allit = [(hp, qb, kc) for qb in range(NQB) for hp in range(HP)
                     for kc in range(NKC)]
            for j in range(PBM_LOOKAHEAD):
                fetch_pbm(*allit[j])

            # ctx matmuls trail the score/exp stream by CTX_LAG iterations
            # ACROSS (hp,qb) segment boundaries, so the new segment's
            # scores never sit behind a ctx mm that waits on the previous
            # pctx bank being evacuated (head-of-line block on the PE)
            pend = []
            pcs = {}

            def emit_ctx(n):
                while len(pend) > n:
                    hp2, qb2, kc2, ex2 = pend.pop(0)
                    if kc2 == 0:
                        pcs[(hp2, qb2)] = [
                            pcx.tile([DHP, 512], F32, tag="pcx",
                                     name=f"pc{b}_{hp2}_{qb2}_{h}")
                            for h in range(2)]
                    pc2 = pcs[(hp2, qb2)]
                    for h in range(2):
                        hh = hp2 * 2 + h
                        mm(pc2[h][:, :],
                           hvT[(b, kc2)][:, hh * DHP:(hh + 1) * DHP],
                           ex2[:, h * 512:(h + 1) * 512],
                           start=kc2 == 0, stop=kc2 == NKC - 1)
                    if kc2 == NKC - 1:
                        t = cxp.tile([DHP, 1024], BF16, tag="cxr",
                                     name=f"cxr{b}_{hp2}_{qb2}")
                        for h in range(2):
                            # ACT evac: keeps these off the DVE, whose tmq
                            # adds gate the score-psum recycling
                            nc.scalar.copy(
                                t[:, h * 512:(h + 1) * 512], pc2[h][:, :])
                        cxr[(b, hp2, qb2)] = t

            for j, (hp, qb, kc) in enumerate(allit):
                if j + PBM_LOOKAHEAD < len(allit):
                    fetch_pbm(*allit[j + PBM_LOOKAHEAD])
                pbm_t = pre.pop((hp, qb, kc))
                # PE-add only for b1: b0's attention overlaps the b1
                # projections (PE-bound there), b1's overlaps the
                # DVE/ACT-heavy out-projections
                use_ident = b == 1
                sc = psc.tile([128, 1024], F32, tag="sc",
                              name=f"sc{b}_{hp}_{qb}_{kc}")
                for h in range(2):
                    hh = hp * 2 + h
                    mm(sc[:, h * 512:(h + 1) * 512],
                       hk_sb[(b, hh)][:, kc * 128:(kc + 1) * 128],
                       hq_sb[(b, hh)][:, qb * 512:(qb + 1) * 512],
                       start=True, stop=not use_ident)
                if use_ident:
                    for h in range(2):
                        mm(sc[:, h * 512:(h + 1) * 512], ident[:, :],
                           pbm_t[:, h * 512:(h + 1) * 512],
                           start=False, stop=True)
                    src = sc
                else:
                    tmq = tmqp.tile([128, 1024], F32, tag="tmq",
                                    name=f"tq{b}_{hp}_{qb}_{kc}")
                    if CROSSBANK_ADD:
                        nc.vector.tensor_add(tmq[:, :], sc[:, :], pbm_t[:, :])
                    else:
                        for h in range(2):
                            s = slice(h * 512, (h + 1) * 512)
                            nc.vector.tensor_add(tmq[:, s], sc[:, s],
                                                 pbm_t[:, s])
                    src = tmq
                ex = expp.tile([128, 1024], BF16, tag="ex",
                               name=f"ex{b}_{hp}_{qb}_{kc}")
                nc.scalar.activation(ex[:, :], src[:, :], Exp)
                pend.append((hp, qb, kc, ex))
                emit_ctx(CTX_LAG)
                yield
            emit_ctx(0)

        # --- normalize cluster -------------------------------------------
        # 1/denom = exp(-ln(denom)) on ACT; the combined ln+exp table is
        # preloaded once at kernel start so these cause NO table swaps
        def norm(b, hpqs):
            for hp in {hp for hp, _ in hpqs}:
                if (b, hp) not in ctxn:
                    ctxn[(b, hp)] = sbp.tile([128, LQ], BF16,
                                             tag=f"ctxn{hp}",
                                             name=f"ctxn{b}_{hp}")
            rls = {}
            for hp, qb in hpqs:
                rl = nrmp.tile([1, 1024], F32, tag="rl",
                               name=f"rl{b}_{hp}_{qb}")
                nc.scalar.activation(rl[:, :], cxr[(b, hp, qb)][DH:DHP, :], Ln)
                rls[(hp, qb)] = rl
            for hp, qb in hpqs:
                t = cxr[(b, hp, qb)]
                rc = nrmp.tile([1, 1024], BF16, tag="rc",
                               name=f"rc{b}_{hp}_{qb}")
                nc.scalar.activation(rc[:, :], rls[(hp, qb)][:, :], Exp,
                                     scale=-1.0)
                bc = nrmp.tile([64, 1024], BF16, tag="bc",
                               name=f"bc{b}_{hp}_{qb}")
                nc.gpsimd.partition_broadcast(bc[:, :], rc[:, :])
                for h in range(2):
                    nc.vector.tensor_mul(
                        ctxn[(b, hp)][h * 64:(h + 1) * 64,
                                      qb * 512:(qb + 1) * 512],
                        t[0:DH, h * 512:(h + 1) * 512],
                        bc[:, h * 512:(h + 1) * 512])

        # --- out-projection ----------------------------------------------
        # per-(qb,oc) [128,512] units, qb-outer (a unit never enters the
        # in-order PE queue before its qb's norms have had time to land).
        # Evac rotates across engines so no single engine gates the tail;
        # per-half DMA spreads the output drain.
        def outproj_units(b, qbs=(0, 1), evacs=("v",)):
            # gpsimd cannot read PSUM on trn2 -> only DVE ("v") and ACT
            # ("s") can evacuate psum tiles
            evac_fn = {"v": nc.vector.tensor_copy,
                       "s": nc.scalar.copy}
            k = 0
            for qb in qbs:
                for oc in range(NOC):
                    po = ppj.tile([128, 512], F32, tag="pj",
                                  name=f"po{b}_{oc}_{qb}")
                    for c in range(NMC):
                        mm(po[:, :], wo_v[c][:, oc * 128:(oc + 1) * 128],
                           ctxn[(b, c)][:, qb * 512:(qb + 1) * 512],
                           start=c == 0, stop=c == NMC - 1)
                    osb = osbp.tile([128, 512], BF16, tag="osb",
                                    name=f"osb{b}_{oc}_{qb}")
                    evac_fn[evacs[k % len(evacs)]](osb[:, :], po[:, :])
                    k += 1
                    nc.sync.dma_start(
                        out_a[b, oc * 128:(oc + 1) * 128,
                              qb * 512:(qb + 1) * 512], osb[:, :])
                    yield

        def outproj_tail(b, qb, evacs=("v", "s")):
            # tail stage: the score-psum pool is free once b's attention is
            # done -> pack 2 oc per [128,1024] psc tile (4 units in flight
            # vs 2 in ppj), one big evac, per-oc DMA
            evac_fn = {"v": nc.vector.tensor_copy, "s": nc.scalar.copy}
            for og in range(NOC // 2):
                po = psc.tile([128, 1024], F32, tag="sc",
                              name=f"pot{b}_{og}_{qb}")
                # c-outer so consecutive matmuls hit ALTERNATE psum banks:
                # back-to-back accumulation into the same bank region has a
                # ~250ns RMW stall (630ns vs 379ns observed)
                for c in range(NMC):
                    for i in range(2):
                        oc = 2 * og + i
                        mm(po[:, i * 512:(i + 1) * 512],
                           wo_v[c][:, oc * 128:(oc + 1) * 128],
                           ctxn[(b, c)][:, qb * 512:(qb + 1) * 512],
                           start=c == 0, stop=c == NMC - 1)
                osb = osbp.tile([128, 1024], BF16, tag="osb2",
                                name=f"osbt{b}_{og}_{qb}")
                evac_fn[evacs[og % len(evacs)]](osb[:, :], po[:, :])
                for i in range(2):
                    oc = 2 * og + i
                    nc.sync.dma_start(
                        out_a[b, oc * 128:(oc + 1) * 128,
                              qb * 512:(qb + 1) * 512],
                        osb[:, i * 512:(i + 1) * 512])
                yield

        # ================== schedule ==================
        # S0: b0 projections, K -> V -> Q (V consumes already-resident hkv
        # while the hq stream arrives; attention needs Q last anyway)
        alloc_hkhq(0)
        proj_b0_dc_outer("k", wk_v, hkv_t0, hk_sb)
        drain(v_units(0, hkv_t0))
        proj_b0_dc_outer("q", wq_v, hq_t0, hq_sb)
        load_wo()
        # S1: b0 attention || b1 projections
        hkv_t1 = fetch_hid("hkvh", hkv_a, 1)
        p1 = proj_units(1, hkv_t1)
        for j, _ in enumerate(attn_iters(0)):
            # backloaded filler: late S1 has no projection slack otherwise
            # and the score->add->exp->ctx chain stalls surface on the PE
            drain(p1, 1 if j < 16 else 2)
            if j == 2 * NKC + CTX_LAG - 1:   # b0 qb0 cxr (both hp) ready
                norm(0, [(0, 0), (1, 0)])    # under S1's proj filler
            if j == 3 * NKC + CTX_LAG - 1:
                norm(0, [(0, 1)])
        drain(p1)
        # S2: b1 attention || b0 out-projection || b1 qb0 out-projection
        o0 = outproj_units(0)
        a1 = attn_iters(1)
        norm(0, [(1, 1)])
        o1q0 = None
        for j, _ in enumerate(a1):
            drain(o0, 1)
            if j == 2 * NKC + CTX_LAG - 1:   # b1 qb0 cxr (both hp) ready
                norm(1, [(0, 0), (1, 0)])
            if j == 2 * NKC + CTX_LAG + 2:   # norm latency padding: the
                o1q0 = outproj_units(1, qbs=(0,),  # PE queue is in-order
                                     evacs=("v", "s"))
            if o1q0 is not None:
                drain(o1q0, 1)
            if j == 3 * NKC + CTX_LAG - 1:
                norm(1, [(0, 1)])
        drain(o0)
        # S3: last norm + b1 qb1 out-projection; leftover o1q0 units are
        # the PE filler under the final norm chain
        norm(1, [(1, 1)])
        t1 = outproj_tail(1, 1)
        more_a, more_b = o1q0 is not None, True
        while more_a or more_b:   # interleave the independent o1q0
            if more_a:            # leftovers with the norm-gated tail
                more_a = not drain(o1q0, 1)
            if more_b:
                more_b = not drain(t1, 1)

    nc.compile()
    return nc


_NC_CACHE = None


def _get_nc():
    global _NC_CACHE
    if _NC_CACHE is None:
        _NC_CACHE = build_nc()
    return _NC_CACHE


def make_in_maps(hidden_q, hidden_kv, mask, position_bias, Wq, Wk, Wv, Wo):
    hidden_q = np.asarray(hidden_q, np.float32)
    hidden_kv = np.asarray(hidden_kv, np.float32)
    mask = np.asarray(mask)
    position_bias = np.asarray(position_bias, np.float32)
    Wq, Wk, Wv, Wo = (np.asarray(w, np.float32) for w in (Wq, Wk, Wv, Wo))

    maskb = np.where(mask != 0, np.float32(0), np.float32(NEG))  # [B, LK, LQ]

    def ilv_hid(x):
        # [B, D, L] -> [B, D/256, 128, 2*L]: partition p of chunk-pair g
        # holds rows g*256+p and g*256+128+p contiguously
        b_, d_, l_ = x.shape
        return np.ascontiguousarray(
            x.reshape(b_, d_ // 256, 2, 128, l_).transpose(0, 1, 3, 2, 4)
        ).reshape(b_, d_ // 256, 128, 2 * l_)

    def ilv_w(wt):
        # [D, MR] -> [D/512, 128, 4*MR]
        d_, m_ = wt.shape
        return np.ascontiguousarray(
            wt.reshape(d_ // 512, 4, 128, m_).transpose(0, 2, 1, 3)
        ).reshape(d_ // 512, 128, 4 * m_)

    f16 = _NP[F16]
    bf16 = _NP[BF16]
    hq = ilv_hid(hidden_q.astype(f16))
    hkv = ilv_hid(hidden_kv.astype(f16))
    ident = np.eye(128, dtype=bf16)
    in_maps = []
    for c in range(NCORES):
        hs = slice(c * HC, (c + 1) * HC)
        rs = slice(c * MR, (c + 1) * MR)
        # pbm[b, hp, kc, qb, p, h01*512+j]
        pbm = (position_bias[hs][None] + maskb[:, None])  # [B, 4, LK, LQ]
        pbm = np.ascontiguousarray(
            pbm.reshape(B, HP, 2, NKC, 128, NQB, 512)
               .transpose(0, 1, 3, 5, 4, 2, 6)
        ).reshape(B, HP, NKC, NQB, 128, 1024).astype(bf16)
        in_maps.append({
            "hq": hq,
            "hkv": hkv,
            "pbm": pbm,
            "wqt": ilv_w(np.ascontiguousarray(Wq[rs].T).astype(f16)),
            "wkt": ilv_w(np.ascontiguousarray(Wk[rs].T).astype(f16)),
            "wvt": ilv_w(np.ascontiguousarray(Wv[rs].T).astype(f16)),
            "wot": np.ascontiguousarray(Wo[:, rs].T).astype(bf16),
            "ident": ident,
        })
    return in_maps


def run(in_maps, trace=False):
    nc = _get_nc()
    return run_bass_kernel_spmd(nc, in_maps, core_ids=list(range(NCORES)),
                                trace=trace)


def kernel(hidden_q, hidden_kv, mask, position_bias, Wq, Wk, Wv, Wo):
    in_maps = make_in_maps(hidden_q, hidden_kv, mask, position_bias,
                           Wq, Wk, Wv, Wo)
    res = run(in_maps, trace=False)
    acc = np.zeros((B, D, LQ), np.float32)
    for r in res.results:
        acc += np.asarray(r["out"], dtype=np.float32)
    return acc

